# revision 7
# baseline (speedup 1.0000x reference)
"""ANIMAZero recurrent cell on 8 TRN2 NeuronCores (Bass/Tile).

Data-parallel: batch 1024 is split into 8 shards of 128; each core runs
the full T=256 recurrence on its shard, split into TWO phase-offset
chains of 64 batch columns. The per-step latency is dominated by the
serial sigma/tanh dependency chain (6 ScalarE ops + ~14 cross-engine
sem hops); running two half-width chains halves each ScalarE/VectorE
op's cost while the chains' ops interleave per engine, hiding most hop
latency and pushing ScalarE/VectorE to ~80% occupancy.

Per step and chain, the three D=32 states [W; I; A] live stacked on
SBUF partitions so each gate group is one fp16 matmul. sigma3x
co-computes multA(t) with next step's attn gate (both contract
[W_new; I_new]), so attn*obs_enc runs off-chain on the idle GpSimd
engine (rotating partition layouts keep it aligned with the obs
encoding). PSUM regions are grouped per chain so every PSUM tile has a
single reader engine: Tile's tile-granular reader/writer tracking then
never couples the chains or puts a false cross-engine wait on the
chain. W_all/A_all are staged to fp16 SBUF on VectorE under the sigma
windows so the gating products run in DVE 2x mode (GpSimd cannot touch
PSUM on hardware). The phi output projection accumulates 4 steps in a
per-chain PSUM region and drains via VectorE + strided DMA.
"""

import os
import sys

sys.path.insert(0, "/opt/trn_rl_repo")
import numpy as np
import bass_rust
import concourse.bass as bass
import concourse.tile as tile
from concourse import mybir

F32 = mybir.dt.float32
F16 = mybir.dt.float16
SIG = mybir.ActivationFunctionType.Sigmoid
TANH = mybir.ActivationFunctionType.Tanh
IDENT = mybir.ActivationFunctionType.Identity
MULT = mybir.AluOpType.mult
ADD = mybir.AluOpType.add
SUB = mybir.AluOpType.subtract

D, S, O, T, B = 32, 8, 4, 256, 1024
N_CORES = 8
BC = B // N_CORES  # 128 batch per core
G = BC
WDT = np.float16

# ---------------------------------------------------------------------------
# walrus in this container rejects instructions carrying more than one sem
# wait ("Too many sync wait commands"). After Tile lowers everything, move
# surplus waits onto same-engine NOPs inserted just before each offender.
_MAXW = 1


def _split_waits(nc):
    for f in nc.m.functions:
        for blk in f.blocks:
            il = blk.instructions
            cur = list(il)
            out_list = []
            changed = False
            for ins in cur:
                si = ins.sync_info
                w = list(si.on_wait or []) if si is not None else []
                if len(w) > _MAXW:
                    changed = True
                    for i in range(0, len(w) - _MAXW, _MAXW):
                        bi = nc.engines[ins.engine].nop(nofuse=True)
                        nop_ins = bi.ins
                        for srch in (blk,) + tuple(f.blocks):
                            lst = srch.instructions
                            if lst and lst[-1] is nop_ins:
                                lst.pop()
                                break
                        nop_ins.sync_info = bass_rust.SyncInfo(
                            on_wait=w[i : i + _MAXW], on_update=[]
                        )
                        out_list.append(nop_ins)
                    si.on_wait = w[len(w) - _MAXW :]
                out_list.append(ins)
            if changed:
                il[:] = out_list


_orig_drain = tile.TileContext._drain_and_barrier


def _drain_then_split(self, tick_clock, wait_clock):
    _orig_drain(self, tick_clock, wait_clock)
    _split_waits(self.nc)


tile.TileContext._drain_and_barrier = _drain_then_split

# ---------------------------------------------------------------------------
# attn for step t+1 (k2 = (t+1)%4) is co-computed with multA in sigma3x; it
# must land at partition k2*32 to align with the obs encoding, and a 64-wide
# matmul output may only start at partition 0 or 64 — hence 4 column-order
# variants of wdm.
ABASE = [0, 32, 64, 96]  # attn partition base per k2
MBASE = [32, 0, 96, 64]  # multA partition base per k2
BASE0 = [0, 0, 64, 64]  # sigma3x window base per k2

WEIGHT_SPECS = [
    ("wam", [96, 32], F16),  # cols: mult_pre (wmg over I;A slots)
    ("waw", [96, 32], F16),  # cols: W_all
    ("wb", [96, 96], F16),  # cols: z | r | multI
    ("wc", [96, 32], F16),  # rows: hW | hI | hA
    ("wda", [97, 32], F16),  # cols: A_all; row 96 = a_b (ones row)
    ("wdm0", [97, 64], F16),  # attn|multA variants per k2 of next step
    ("wdm1", [97, 64], F16),
    ("wdm2", [97, 64], F16),
    ("wdm3", [97, 64], F16),
    ("wphi", [97, 4], F16),  # rows 64:96 = phi_w.T; row 96 = phi_b (ones row)
    ("wenc", [32, 128], F16),  # 4x block-diag enc_w.T
    ("biases", [128, 12], F32),
]


def _pack_weights(inp):
    g = {k: np.ascontiguousarray(np.asarray(v, np.float32)) for k, v in inp.items()}

    wam = np.zeros((96, 32), np.float32)
    wam[32:64] = g["wmg_w"][:, 0:32].T
    wam[64:96] = g["wmg_w"][:, 32:64].T
    waw = np.concatenate([g["wW"].T, g["wI"].T, g["wA"].T], axis=0)

    zb = np.concatenate([g["zW"].T, g["zI"].T, g["zA"].T], axis=0)
    rb = np.concatenate([g["rW"].T, g["rI"].T, g["rA"].T], axis=0)
    mib = np.zeros((96, 32), np.float32)
    mib[0:32] = g["img_w"][:, 0:32].T
    mib[64:96] = g["img_w"][:, 32:64].T
    wb = np.concatenate([zb, rb, mib], axis=1)

    wc = np.concatenate([g["hW"].T, g["hI"].T, g["hA"].T], axis=0)

    wda = np.concatenate(
        [g["aW"].T, g["aI"].T, g["aA"].T, g["a_b"][None, :]], axis=0
    )
    attb = np.zeros((97, 32), np.float32)
    attb[0:32] = g["att_w"][:, 0:32].T
    attb[32:64] = g["att_w"][:, 32:64].T
    amgb = np.zeros((97, 32), np.float32)
    amgb[0:32] = g["amg_w"][:, 0:32].T
    amgb[32:64] = g["amg_w"][:, 32:64].T
    wdm = {}
    for k2 in range(4):
        if ABASE[k2] < MBASE[k2]:
            wdm[f"wdm{k2}"] = np.concatenate([attb, amgb], axis=1)
        else:
            wdm[f"wdm{k2}"] = np.concatenate([amgb, attb], axis=1)

    wphi = np.zeros((97, 4), np.float32)
    wphi[64:96] = g["phi_w"].T
    wphi[96] = g["phi_b"]

    wenc = np.zeros((32, 128), np.float32)
    for k in range(4):
        wenc[k * 8 : (k + 1) * 8, k * 32 : (k + 1) * 32] = g["enc_w"].T

    biases = np.zeros((128, 12), np.float32)
    biases[0:32, 0] = g["wmg_b"]
    biases[0:32, 1] = g["z_b"]
    biases[32:64, 1] = g["r_b"]
    biases[64:96, 1] = g["img_b"]
    biases[0:32, 2] = g["h_b"]
    biases[0:32, 4] = g["att_b"]  # for the t=0 attn bootstrap
    biases[:, 5] = np.tile(g["enc_b"], 4)
    for k2 in range(4):  # sigma3x bias: att_b and amg_b at layout positions
        biases[ABASE[k2] : ABASE[k2] + 32, 8 + k2] = g["att_b"]
        biases[MBASE[k2] : MBASE[k2] + 32, 8 + k2] = g["amg_b"]

    w = dict(
        wam=wam, waw=waw, wb=wb, wc=wc, wda=wda, wphi=wphi, wenc=wenc,
        biases=biases, **wdm,
    )
    return {
        k: np.ascontiguousarray(v if k == "biases" else v.astype(WDT))
        for k, v in w.items()
    }


def _pack_obs_shard(obs_shard):
    """[T, BC, S] f32 -> [32, T/4*BC] fp16: row k*8+s, col c*BC+b holds
    obs[4c+k, b, s] (4 timesteps stacked on partitions)."""
    x = np.ascontiguousarray(obs_shard).reshape(T // 4, 4, BC, S)
    x = x.transpose(1, 3, 0, 2)
    return np.ascontiguousarray(x.reshape(32, (T // 4) * BC)).astype(WDT)


def _unpack_out(out_core):
    """[4, T/4, 4, BC] (or [4, T*BC]) -> [T, BC, O]."""
    return np.ascontiguousarray(
        np.asarray(out_core).reshape(O, T, BC).transpose(1, 2, 0)
    )


def _build_nc():
    nc = bass.Bass()
    obs4 = nc.declare_dram_parameter("obs4", [32, (T // 4) * BC], F16, isOutput=False)
    wdram = {}
    for name, shape, dt in WEIGHT_SPECS:
        wdram[name] = nc.declare_dram_parameter(name, shape, dt, isOutput=False)
    out = nc.declare_dram_parameter("out", [4, T // 4, 4, BC], F32, isOutput=True)

    with tile.TileContext(nc) as tc:
        with (
            tc.tile_pool(name="singles", bufs=1) as singles,
            tc.tile_pool(name="psum", bufs=1, space="PSUM") as psum,
            tc.tile_pool(name="outp", bufs=3) as outp,
        ):
            wsb = {}
            for name, shape, dt in WEIGHT_SPECS:
                wsb[name] = singles.tile(shape, dt, name=f"w_{name}")
                nc.sync.dma_start(out=wsb[name], in_=wdram[name][:, :])
            obs_sb = singles.tile([32, (T // 4) * BC], F16)
            nc.sync.dma_start(out=obs_sb, in_=obs4[:, :])

            bia = wsb["biases"]

            # obs_enc_all = tanh(wenc.T @ obs4 + enc_b), all steps up front
            oenc = singles.tile([128, (T // 4) * BC], F16)
            NPRE = (T // 4) * BC // 512
            with tc.tile_pool(name="psum_pre", bufs=1, space="PSUM") as psum_pre:
                for i in range(NPRE):
                    ppre = psum_pre.tile([128, 512], F32)
                    nc.tensor.matmul(
                        ppre, wsb["wenc"], obs_sb[:, i * 512 : (i + 1) * 512],
                        start=True, stop=True,
                    )
                    nc.scalar.activation(
                        out=oenc[:, i * 512 : (i + 1) * 512], in_=ppre,
                        func=TANH, bias=bia[:, 5:6],
                    )

            # SB-SB elementwise inputs must share a start partition; outputs
            # are free. fp16 SBUF ops hit the DVE 2x mode. Two phase-locked
            # chains of 64 batch columns each run the recurrence; their ops
            # interleave pairwise per engine so each chain's cross-engine sem
            # latency hides under the other chain's execution. Chain-written
            # tiles whose previous-step readers are cross-engine are
            # double-buffered so Tile's conservative WAR waits resolve early.
            NCH = 2
            G2 = G // NCH

            def chain_tiles(h):
                d = {}
                s = lambda nm, shape: singles.tile(shape, F16, name=f"{nm}_{h}")
                d["g2s"] = s("g2s", [96, G2])
                d["ht"] = s("ht", [96, G2])
                d["icp2"] = s("icp2", [32, G2])  # I snapshot (Pool-written)
                d["uneg"] = s("uneg", [32, G2])  # (z-1)*I
                d["vt"] = s("vt", [32, G2])  # z*h*multI
                d["hmt"] = s("hmt", [32, G2])
                d["gmt"] = [s(f"gmt{i}", [32, G2]) for i in range(2)]
                d["xat"] = s("xat", [128, G2])
                d["xbt"] = s("xbt", [128, G2])
                d["wpt"] = [s(f"wpt{i}", [128, G2]) for i in range(2)]
                d["g3x"] = [s(f"g3x{i}", [128, G2]) for i in range(2)]
                d["pat"] = [s(f"pat{i}", [32, G2]) for i in range(2)]
                d["stk"] = s("stk", [128, G2])
                nc.vector.memset(d["stk"], 0.0)
                nc.vector.memset(d["stk"][96:97, :], 1.0)  # ones row
                nc.vector.memset(d["icp2"], 0.0)  # I(-1) = 0
                return d

            # Per-chain PSUM, 3 banks each: regions that share a tile have
            # all their engine-readers on ONE engine, so Tile's tile-granular
            # last-writer/reader tracking never couples the two chains or
            # puts a false cross-engine wait on the chain.
            CH = [chain_tiles(h) for h in range(NCH)]
            for h in range(NCH):
                d = CH[h]
                tA = psum.tile([128, 2 * G2], F32, name=f"psA_{h}")  # ACT-read
                tB = psum.tile([96, 2 * G2], F32, name=f"psB_{h}")  # ACT-read
                tC = psum.tile([128, 7 * G2], F32, name=f"psC_{h}")  # DVE-read
                d["pmx"] = tA[:, 0:G2]
                d["p0m"] = tA[0:32, G2 : 2 * G2]
                d["p1"] = tB[:, 0:G2]
                d["p2h"] = tB[0:32, G2 : 2 * G2]
                d["p0w"] = tC[0:32, 0:G2]
                d["p3aT"] = tC[:, G2 : 2 * G2]  # A_all, partition base rotates
                d["p4"] = tC[0:4, 3 * G2 : 7 * G2]
            zz = singles.tile([32, G2], F16, name="zz")
            at0 = singles.tile([32, G2], F16, name="at0")
            nc.vector.memset(zz, 0.0)

            def emit_phi(h, t):
                # phi matmul for step t (phi_b rides the ones row), deferred
                # into step t+1's sigma-m window.
                d = CH[h]
                nc.tensor.matmul(
                    d["p4"][0:4, (t % 4) * G2 : (t % 4 + 1) * G2],
                    wsb["wphi"][64:97, :], d["stk"][64:97, :],
                    start=True, stop=True,
                )

            def emit_evac(h, t):
                # evacuate on DVE (GPSIMD cannot access PSUM on hardware),
                # emitted after the gpa pair so it runs in the tanhA window.
                if t < 0 or t % 4 != 3:
                    return
                d = CH[h]
                ch = outp.tile([4, 4 * G2], F32)
                nc.vector.tensor_copy(out=ch, in_=d["p4"])
                nc.sync.dma_start(
                    out=out[0:4, t // 4, 0:4, h * G2 : (h + 1) * G2], in_=ch
                )

            # t=0 attn bootstrap: attn(0) = sigmoid(att_b), xa0 = attn0*oe0
            nc.scalar.activation(out=at0, in_=zz, func=SIG, bias=bia[0:32, 4:5])
            for h in range(NCH):
                nc.vector.tensor_tensor(
                    out=CH[h]["xat"][0:32, :], in0=at0,
                    in1=oenc[0:32, h * G2 : (h + 1) * G2], op=MULT,
                )

            def oe_slice(h, t):
                c, k = t // 4, t % 4
                return oenc[
                    k * 32 : (k + 1) * 32,
                    c * BC + h * G2 : c * BC + (h + 1) * G2,
                ]

            def emit_A(h, t):
                k = t % 4
                d = CH[h]
                if t == 0:
                    # W_all(0) = 0, so W_new(0) = tanh(attn0 * oe0)
                    nc.scalar.activation(
                        out=d["stk"][0:32, :], in_=d["xat"][0:32, :], func=TANH
                    )
                    return
                # I(t-1) snapshot on Pool, issued early: its RAW (prev
                # I_new) resolved long ago, and the WAR it imposes on this
                # step's r*I in-place clobber resolves well before sigma3.
                nc.gpsimd.tensor_copy(out=d["icp2"], in_=d["stk"][32:64, :])
                s96 = d["stk"][0:96, :]
                nc.tensor.matmul(d["p0m"], wsb["wam"], s96, start=True, stop=True)
                nc.tensor.matmul(d["p0w"], wsb["waw"], s96, start=True, stop=True)
                emit_phi(h, t - 1)  # runs on PE during sigma-m
                nc.scalar.activation(
                    out=d["gmt"][t % 2], in_=d["p0m"], func=SIG, bias=bia[0:32, 0:1]
                )
                xa = d["xat"][k * 32 : k * 32 + 32, :]
                xb = d["xbt"][k * 32 : k * 32 + 32, :]
                wpre = d["wpt"][t % 2][k * 32 : k * 32 + 32, :]
                # gate product reads W_all straight from PSUM (in0/in1 share
                # start partition 0; out base is free)
                nc.vector.tensor_tensor(out=xb, in0=d["gmt"][t % 2], in1=d["p0w"], op=MULT)
                nc.vector.tensor_tensor(out=wpre, in0=xa, in1=xb, op=ADD)
                nc.scalar.activation(out=d["stk"][0:32, :], in_=wpre, func=TANH)

            def emit_B(h, t):
                d = CH[h]
                gg2 = d["g2s"]
                nc.tensor.matmul(d["p1"], wsb["wb"], d["stk"][0:96, :], start=True, stop=True)
                nc.scalar.activation(out=gg2, in_=d["p1"], func=SIG, bias=bia[0:96, 1:2])
                # r*I overwrites the I-slot in place (Pool snapshotted I in
                # emit_A) so the h matmul is one contiguous K=96 contraction.
                nc.vector.tensor_tensor(out=d["stk"][32:64, :], in0=gg2[32:64, :], in1=d["stk"][32:64, :], op=MULT)
                nc.tensor.matmul(d["p2h"], wsb["wc"], d["stk"][0:96, :], start=True, stop=True)
                # uneg = (z-1)*I in one fused op (z and icp2 both base 0)
                nc.vector.scalar_tensor_tensor(
                    out=d["uneg"], in0=gg2[0:32, :], scalar=1.0, in1=d["icp2"],
                    op0=SUB, op1=MULT,
                )
                nc.scalar.activation(out=d["ht"][64:96, :], in_=d["p2h"], func=TANH, bias=bia[0:32, 2:3])
                emit_evac(h, t - 1)  # DVE, lands in the tanhH window
                gh = d["ht"][64:96, :]
                # hm = h*multI (both base 64), v = hm*z (both base 0),
                # I_new = v - uneg (both base 0): no alignment copies needed.
                nc.vector.tensor_tensor(out=d["hmt"], in0=gh, in1=gg2[64:96, :], op=MULT)
                nc.vector.tensor_tensor(out=d["vt"], in0=d["hmt"], in1=gg2[0:32, :], op=MULT)
                nc.vector.tensor_tensor(out=d["stk"][32:64, :], in0=d["vt"], in1=d["uneg"], op=SUB)

            def emit_C(h, t):
                # sigma3x computes multA(t) AND attn(t+1) in one op (same
                # [W_new; I_new] contraction); a_b rides the ones row
                # through wda.
                k2 = (t + 1) % 4
                b0, ab, mb = BASE0[k2], ABASE[k2], MBASE[k2]
                d = CH[h]
                s97 = d["stk"][0:97, :]
                nc.tensor.matmul(
                    d["pmx"][b0 : b0 + 64, :], wsb[f"wdm{k2}"], s97,
                    start=True, stop=True, tile_position=(0, b0),
                )
                # A_all lands at partition mb so the gating product's PSUM
                # operand shares multA's start partition
                p3a = d["p3aT"][mb : mb + 32, :]
                nc.tensor.matmul(p3a, wsb["wda"], s97, start=True, stop=True,
                                 tile_position=(0, mb))
                nc.scalar.activation(
                    out=d["g3x"][t % 2][b0 : b0 + 64, :], in_=d["pmx"][b0 : b0 + 64, :],
                    func=SIG, bias=bia[b0 : b0 + 64, 8 + k2 : 9 + k2],
                )
                nc.vector.tensor_tensor(out=d["pat"][t % 2], in0=d["g3x"][t % 2][mb : mb + 32, :], in1=p3a, op=MULT)
                if t + 1 < T:
                    # xa for step t+1 on Pool: after gpa in the g3x reader
                    # chain and outside the DVE counter.
                    nc.gpsimd.tensor_tensor(
                        out=d["xat"][k2 * 32 : k2 * 32 + 32, :],
                        in0=d["g3x"][t % 2][ab : ab + 32, :],
                        in1=oe_slice(h, t + 1), op=MULT,
                    )
                nc.scalar.activation(out=d["stk"][64:96, :], in_=d["pat"][t % 2], func=TANH)  # A_new

            # Chain 1 is emitted ~half a step behind chain 0 so its ACT ops
            # fill the gaps left by chain 0's cross-engine excursions.
            for t in range(T):
                emit_A(0, t)
                if t > 0:
                    emit_C(1, t - 1)
                emit_B(0, t)
                emit_A(1, t)
                emit_C(0, t)
                emit_B(1, t)
            emit_C(1, T - 1)
            for h in range(NCH):
                emit_phi(h, T - 1)
            for h in range(NCH):
                emit_evac(h, T - 1)
    return nc


_NC_CACHE = None


def kernel(**inputs):
    global _NC_CACHE
    from concourse.bass_utils import run_bass_kernel_spmd

    obs = np.ascontiguousarray(np.asarray(inputs["obs"], np.float32))
    w = _pack_weights({k: v for k, v in inputs.items() if k != "obs"})

    if _NC_CACHE is None:
        _NC_CACHE = _build_nc()
    nc = _NC_CACHE

    in_maps = []
    for i in range(N_CORES):
        m = dict(w)
        m["obs4"] = _pack_obs_shard(obs[:, i * BC : (i + 1) * BC, :])
        in_maps.append(m)

    res = run_bass_kernel_spmd(
        nc, in_maps, core_ids=list(range(N_CORES)), trace=False
    )
    outs = [_unpack_out(np.asarray(res.results[i]["out"])) for i in range(N_CORES)]
    return np.concatenate(outs, axis=1).astype(np.float32)  # [T, B, O]

